# revision 9
# baseline (speedup 1.0000x reference)
"""Trainium2 Bass kernel for windowed ViT attention with decomposed relative
position bias (B=8, N=1024=32x32, C=768, 12 heads, head_dim 64).

Sharding: data-parallel over batch B across 8 NeuronCores (1 image per core).

v4 = v3 + cross-rep software pipelining:
  - Persistent PSUM pools with fixed tags: sps 3x[128,512] (exp runs on
    half-chunks), ops 2x[65,1024] (o_ps accumulators; yp borrows the tag
    in phase E), aux 1x[128,512] (all projection accumulators) = 8 banks.
  - qaug / vaug / atile double-buffered by rep parity; kasm single
    (fine-grained AP deps + emission placement avoid WAR stalls).
  - Projection work (q/rel/v/k) of rep N+1 is emitted in small blocks
    interleaved into rep N's attention loop, so PE fills the ACT-bound
    stretches and the den-drain tail; per-rep steady state is PE-bound.
  - den path straight from PSUM: reciprocal of the ones-row, gpsimd
    partition-broadcast, DVE normalize into atile.  All-bf16 operand path.
"""

import sys

if "/opt/trn_rl_repo" not in sys.path:
    sys.path.insert(0, "/opt/trn_rl_repo")

import numpy as np

NUM_HEADS = 12
N_CTX = 1024
C_DIM = 768
HD = 64
HH = 32
NCORES = 8

_CACHE: dict = {}


def _build_nc(reps=1, d_heads=12):
    import concourse.mybir as mybir
    import concourse.tile as tile
    from concourse import bacc
    from contextlib import ExitStack

    f32 = mybir.dt.float32
    bf16 = mybir.dt.bfloat16
    Exp = mybir.ActivationFunctionType.Exp

    nc = bacc.Bacc("TRN2", target_bir_lowering=False, debug=False)

    def mm(out, lhsT, rhs, **kw):
        nc.tensor.matmul(out, lhsT, rhs, **kw)

    xt = nc.dram_tensor("xt", [768, 1024], bf16, kind="ExternalInput").ap()
    wqk = nc.dram_tensor("wqk", [768, 1536], bf16, kind="ExternalInput").ap()
    wv = nc.dram_tensor("wv", [768, 768], bf16, kind="ExternalInput").ap()
    wp = nc.dram_tensor("wp", [768, 768], bf16, kind="ExternalInput").ap()
    bias = nc.dram_tensor("bias", [128, 768], f32, kind="ExternalInput").ap()
    ind = nc.dram_tensor("ind", [64, 1024], bf16, kind="ExternalInput").ap()
    rfh = nc.dram_tensor("rfh", [64, 63], bf16, kind="ExternalInput").ap()
    rfw = nc.dram_tensor("rfw", [64, 63], bf16, kind="ExternalInput").ap()
    y = nc.dram_tensor("y", [1024, 768], f32, kind="ExternalOutput").ap()

    with tile.TileContext(nc) as tc, ExitStack() as es:
        singles = es.enter_context(tc.tile_pool(name="singles", bufs=1))

        # double-buffered by rep parity
        qaug_b = [singles.tile([128, 12 * 1024], bf16, name=f"qaug{_i}") for _i in range(2)]
        vaug_b = [singles.tile([128, 8, 12, 65], bf16, name=f"vaug{_i}") for _i in range(2)]
        atile_b = [singles.tile([128, 6, 1024], bf16, name=f"atile{_i}") for _i in range(2)]
        rfh_sb = singles.tile([64, 63], bf16)
        rfw_sb = singles.tile([64, 63], bf16)
        # S_T lhsT tiles: rows 0:64 = kT chunk, 64:128 = indicator rows.
        kasm = singles.tile([128, 6, 2, 8, 128], bf16)
        xt_sb = singles.tile([128, 6, 1024], bf16)
        wq_sb = singles.tile([128, 6, 768], bf16)
        wk_sb = singles.tile([128, 6, 768], bf16)
        wv_sb = singles.tile([128, 6, 768], bf16)
        wp_sb = singles.tile([128, 6, 768], bf16)
        bias_sb = singles.tile([128, 768], f32)

        xt_r = xt.rearrange("(ko p) n -> p ko n", p=128)
        wqk_r = wqk.rearrange("(ko p) n -> p ko n", p=128)
        wv_r = wv.rearrange("(ko p) n -> p ko n", p=128)
        wp_r = wp.rearrange("(ko p) n -> p ko n", p=128)

        # ---- one-time loads ----
        with ExitStack() as es0:
            init = es0.enter_context(tc.tile_pool(name="init", bufs=1))
            ind_sb = init.tile([64, 1024], bf16)
            nc.gpsimd.dma_start(rfh_sb, rfh)
            nc.gpsimd.dma_start(rfw_sb, rfw)
            nc.gpsimd.dma_start(ind_sb, ind)
            for k in range(0, 6, 2):
                nc.sync.dma_start(xt_sb[:, k], xt_r[:, k])
                nc.gpsimd.dma_start(xt_sb[:, k + 1], xt_r[:, k + 1])
            for k in range(0, 6, 2):
                nc.sync.dma_start(wq_sb[:, k], wqk_r[:, k, 0:768])
                nc.gpsimd.dma_start(wq_sb[:, k + 1], wqk_r[:, k + 1, 0:768])
            for k in range(0, 6, 2):
                nc.sync.dma_start(wk_sb[:, k], wqk_r[:, k, 768:1536])
                nc.gpsimd.dma_start(wk_sb[:, k + 1], wqk_r[:, k + 1, 768:1536])
            for k in range(6):
                nc.gpsimd.dma_start(wv_sb[:, k], wv_r[:, k])
                nc.gpsimd.dma_start(wp_sb[:, k], wp_r[:, k])
            nc.gpsimd.dma_start(bias_sb, bias)
            for t in range(6):
                for p in range(2):
                    nc.vector.tensor_copy(
                        kasm[64:128, t, p],
                        ind_sb.rearrange("p (c n) -> p c n", c=8),
                    )
            for v2 in range(2):
                nc.vector.memset(vaug_b[v2][:, :, :, 64:65], 1.0)
            warm = init.tile([1, 1], f32)
            nc.vector.memset(warm, 0.0)
            nc.scalar.activation(warm, warm, Exp)

        # ---- fast prologue: rep-0 projections with dedicated (transient)
        # PSUM pools, closed before the persistent pools open ----
        def bc_fast(rep):
            from contextlib import ExitStack as _ES

            qaug = qaug_b[rep % 2]
            vaug = vaug_b[rep % 2]
            qaug4d = qaug.rearrange("p (hd a b) -> p hd a b", hd=12, a=32)
            with _ES() as esB:
                esQ = esB.enter_context(_ES())
                bqk = esQ.enter_context(tc.tile_pool(name="bqk", bufs=2, space="PSUM"))
                for m in range(6):
                    for n in range(2):
                        ps = bqk.tile([128, 512], f32)
                        for k in range(6):
                            mm(ps, wq_sb[:, k, m * 128 : (m + 1) * 128],
                               xt_sb[:, k, n * 512 : (n + 1) * 512],
                               start=(k == 0), stop=(k == 5))
                        for half, hd in ((0, 2 * m), (64, 2 * m + 1)):
                            dst = qaug[0:64, hd * 1024 + n * 512 : hd * 1024 + (n + 1) * 512]
                            if m % 2 == 0:
                                nc.scalar.copy(dst, ps[half : half + 64, :])
                            else:
                                nc.vector.tensor_copy(dst, ps[half : half + 64, :])
                esQ.close()
                bv = esB.enter_context(tc.tile_pool(name="bv", bufs=2, space="PSUM"))
                esRel = esB.enter_context(_ES())
                cps = esRel.enter_context(tc.tile_pool(name="cps", bufs=2, space="PSUM"))
                for hh in range(0, 32, 2):
                    pg = cps.tile([32, 2, 512], f32, tag="cps")
                    pgv = pg[:, :, 0:384].rearrange("p a (c b) -> p a c b", c=12)
                    for i in range(2):
                        mm(pg[:, i, 0:384].rearrange("p (c b) -> p c b", c=12),
                           rfh_sb[:, 31 - hh - i : 63 - hh - i],
                           qaug4d[0:64, :, hh + i, :], start=True, stop=True)
                    dst = qaug4d[64:96, :, hh : hh + 2, :].rearrange("p c a b -> p a c b")
                    if hh % 4 == 0:
                        nc.vector.tensor_copy(dst, pgv)
                    else:
                        nc.scalar.copy(dst, pgv)
                for ww in range(0, 32, 2):
                    pg = cps.tile([32, 2, 512], f32, tag="cps")
                    pgv = pg[:, :, 0:384].rearrange("p a (c b) -> p a c b", c=12)
                    for j in range(2):
                        mm(pg[:, j, 0:384].rearrange("p (c b) -> p c b", c=12),
                           rfw_sb[:, 31 - ww - j : 63 - ww - j],
                           qaug4d[0:64, :, :, ww + j], start=True, stop=True)
                    dst = qaug4d[96:128, :, :, ww : ww + 2].rearrange("p c b a -> p a c b")
                    if ww % 4 == 0:
                        nc.vector.tensor_copy(dst, pgv)
                    else:
                        nc.scalar.copy(dst, pgv)
                for ch in range(8):
                    pv = bv.tile([128, 768], f32)
                    for c0, cw in ((0, 512), (512, 256)):
                        for k in range(6):
                            mm(pv[:, c0 : c0 + cw],
                               xt_sb[:, k, ch * 128 : (ch + 1) * 128],
                               wv_sb[:, k, c0 : c0 + cw],
                               start=(k == 0), stop=(k == 5))
                    nc.vector.tensor_copy(
                        vaug[:, ch, :, 0:64], pv.rearrange("p (h d) -> p h d", h=12))
                esRel.close()
                kpp = esB.enter_context(tc.tile_pool(name="kpp", bufs=4, space="PSUM"))
                for t in range(6):
                    kp = [kpp.tile([128, 512], f32, name=f"kp{n}", tag="kp")
                          for n in range(2)]
                    for n in range(2):
                        for k in range(6):
                            mm(kp[n], wk_sb[:, k, t * 128 : (t + 1) * 128],
                               xt_sb[:, k, n * 512 : (n + 1) * 512],
                               start=(k == 0), stop=(k == 5))
                        kp4 = kp[n].rearrange("p (c n2) -> p c n2", c=4)
                        for p in range(2):
                            dst = kasm[0:64, t, p, 4 * n : 4 * n + 4]
                            if n == 0:
                                nc.vector.tensor_copy(dst, kp4[64 * p : 64 * p + 64])
                            else:
                                nc.scalar.copy(dst, kp4[64 * p : 64 * p + 64])

        bc_fast(0)

        # ---- persistent PSUM pools (tags fix the bank budget: 3+4+1=8) ----
        sps = es.enter_context(tc.tile_pool(name="sps", bufs=2, space="PSUM"))
        ops = es.enter_context(tc.tile_pool(name="ops", bufs=2, space="PSUM"))
        aux = es.enter_context(tc.tile_pool(name="aux", bufs=2, space="PSUM"))
        expp = es.enter_context(tc.tile_pool(name="expp", bufs=4))
        recp = es.enter_context(tc.tile_pool(name="recp", bufs=2))
        epool = es.enter_context(tc.tile_pool(name="epool", bufs=2))

        # ---- projection block list for one rep (written to parity buffers) --
        def bc_blocks(rep):
            qaug = qaug_b[rep % 2]
            vaug = vaug_b[rep % 2]
            qaug4d = qaug.rearrange("p (hd a b) -> p hd a b", hd=12, a=32)
            blocks = []

            def q_block(m, n):
                def go():
                    ps = aux.tile([128, 512], f32, tag="aux", name="qps")
                    for k in range(6):
                        mm(ps, wq_sb[:, k, m * 128 : (m + 1) * 128],
                           xt_sb[:, k, n * 512 : (n + 1) * 512],
                           start=(k == 0), stop=(k == 5))
                    for half, hd in ((0, 2 * m), (64, 2 * m + 1)):
                        dst = qaug[0:64, hd * 1024 + n * 512 : hd * 1024 + (n + 1) * 512]
                        nc.vector.tensor_copy(dst, ps[half : half + 64, :])
                return go

            def rel_block(axis, i0):
                def go():
                    pg = aux.tile([32, 384], f32, tag="aux", name="rel")
                    pg3 = pg.rearrange("p (c b) -> p c b", c=12)
                    if axis == 0:
                        mm(pg3, rfh_sb[:, 31 - i0 : 63 - i0],
                           qaug4d[0:64, :, i0, :], start=True, stop=True)
                        dst = qaug4d[64:96, :, i0, :]
                    else:
                        mm(pg3, rfw_sb[:, 31 - i0 : 63 - i0],
                           qaug4d[0:64, :, :, i0], start=True, stop=True)
                        dst = qaug4d[96:128, :, :, i0]
                    nc.vector.tensor_copy(dst, pg3)
                return go

            def v_block(ch, part):
                c0, cw = ((0, 512), (512, 256))[part]
                def go():
                    pv = aux.tile([128, cw], f32, tag="aux", name="vps")
                    for k in range(6):
                        mm(pv, xt_sb[:, k, ch * 128 : (ch + 1) * 128],
                           wv_sb[:, k, c0 : c0 + cw],
                           start=(k == 0), stop=(k == 5))
                    dst = vaug[:, ch, c0 // 64 : (c0 + cw) // 64, 0:64]
                    nc.vector.tensor_copy(
                        dst, pv.rearrange("p (h d) -> p h d", d=64))
                return go

            def k_block(t, n):
                def go():
                    kp = aux.tile([128, 512], f32, tag="aux", name="kps")
                    for k in range(6):
                        mm(kp, wk_sb[:, k, t * 128 : (t + 1) * 128],
                           xt_sb[:, k, n * 512 : (n + 1) * 512],
                           start=(k == 0), stop=(k == 5))
                    kp4 = kp.rearrange("p (c n2) -> p c n2", c=4)
                    for p in range(2):
                        dst = kasm[0:64, t, p, 4 * n : 4 * n + 4]
                        nc.vector.tensor_copy(dst, kp4[64 * p : 64 * p + 64])
                return go

            for m in range(6):
                for n in range(2):
                    blocks.append(q_block(m, n))
            for i0 in range(32):
                blocks.append(rel_block(0, i0))
                blocks.append(rel_block(1, i0))
            for ch in range(8):
                for part in range(2):
                    blocks.append(v_block(ch, part))
            for t in range(6):
                for n in range(2):
                    blocks.append(k_block(t, n))
            return blocks

        def emit_phase_d(rep, next_blocks):
            qaug = qaug_b[rep % 2]
            vaug = vaug_b[rep % 2]
            atile = atile_b[rep % 2]
            o_ps_h = {}
            rec_h = {}
            nits = d_heads + 2
            # spread next rep's projection blocks over iterations 1..nits-1
            sched = [[] for _ in range(nits)]
            if next_blocks:
                nb = len(next_blocks)
                slots = nits - 1
                per = (nb + slots - 1) // slots
                for i, blk in enumerate(next_blocks):
                    sched[1 + min(i // per, slots - 1)].append(blk)

            for it in range(nits):
                # prefetch next rep's xt (its last readers ran during rep-1)
                if it == 0 and next_blocks:
                    for k in range(0, 6, 2):
                        nc.sync.dma_start(xt_sb[:, k], xt_r[:, k])
                        nc.gpsimd.dma_start(xt_sb[:, k + 1], xt_r[:, k + 1])

                # stage A (head=it-1): reciprocal of den row + broadcast
                if 0 <= it - 1 < d_heads:
                    hd = it - 1
                    o_ps = o_ps_h[hd]
                    rec = recp.tile([1, 2, 512], f32, tag="rc")
                    nc.vector.reciprocal(rec, o_ps[64:65])
                    rep_t = recp.tile([64, 2, 512], f32, tag="bc")
                    rec_h[hd] = rep_t
                    nc.gpsimd.partition_broadcast(rep_t, rec)

                # stage B (head=it-2): normalize straight from PSUM into atile
                if 0 <= it - 2 < d_heads:
                    hd = it - 2
                    t3 = hd // 2
                    half3 = (hd % 2) * 64
                    rep_t = rec_h.pop(hd)
                    o_ps = o_ps_h.pop(hd)
                    a3 = atile[half3 : half3 + 64, t3, :].rearrange(
                        "p (a b) -> p a b", a=2)
                    nc.vector.tensor_mul(a3, o_ps[0:64], rep_t)

                # stage 0 (head=it): attention chunks in half-chunk units
                if it < d_heads:
                    hd = it
                    par = hd % 2
                    t = hd // 2
                    o_ps = ops.tile([65, 2, 512], f32, name="ops", tag="ops")
                    o_ps_h[hd] = o_ps
                    for ch in range(8):
                        for nt in range(2):
                            s_ps = sps.tile([128, 512], f32, tag="sps")
                            mm(s_ps, kasm[:, t, par, ch],
                               qaug[:, hd * 1024 + nt * 512 : hd * 1024 + (nt + 1) * 512],
                               start=True, stop=True)
                            ex = expp.tile([128, 512], bf16)
                            nc.scalar.activation(ex, s_ps, Exp)
                            mm(o_ps[:, nt], vaug[:, ch, hd, :], ex,
                               start=(ch == 0), stop=(ch == 7))

                # interleaved projection blocks for rep+1
                for blk in sched[it]:
                    blk()

        def emit_phase_e(rep):
            atile = atile_b[rep % 2]
            for ch in range(8):
                yp = ops.tile([128, 768], f32, tag="ops", name="yp")
                for k in range(6):
                    for c0, cw in ((0, 512), (512, 256)):
                        mm(yp[:, c0 : c0 + cw],
                           atile[:, k, ch * 128 : (ch + 1) * 128],
                           wp_sb[:, k, c0 : c0 + cw],
                           start=(k == 0), stop=(k == 5))
                y_sb = epool.tile([128, 768], f32)
                nc.vector.tensor_add(y_sb, yp, bias_sb)
                nc.sync.dma_start(y[ch * 128 : (ch + 1) * 128, :], y_sb)

        for _rep in range(reps):
            nxt = bc_blocks(_rep + 1) if _rep + 1 < reps else None
            emit_phase_d(_rep, nxt)
            emit_phase_e(_rep)

    nc.compile()
    return nc


def _host_prep(qkv_w, rel_pos_h, rel_pos_w, proj_w, proj_b):
    import ml_dtypes

    bf16 = ml_dtypes.bfloat16
    qkv_w = np.asarray(qkv_w, np.float32)
    scale = 1.0 / np.sqrt(HD)
    wqk = np.ascontiguousarray(qkv_w[0:1536].T)  # [768, 1536]
    wqk[:, 0:768] *= scale
    wv = np.ascontiguousarray(qkv_w[1536:2304].T).astype(bf16)  # [768, 768]
    wp = np.ascontiguousarray(np.asarray(proj_w, np.float32).T).astype(bf16)
    bias = np.ascontiguousarray(
        np.broadcast_to(np.asarray(proj_b, np.float32)[None, :], (128, 768))
    )
    k2 = np.arange(1024)
    indm = np.zeros((64, 1024), np.float32)
    indm[0:32] = (k2[None, :] // 32) == np.arange(32)[:, None]
    indm[32:64] = (k2[None, :] % 32) == np.arange(32)[:, None]
    rfh = np.ascontiguousarray(np.asarray(rel_pos_h, np.float32)[::-1].T).astype(bf16)
    rfw = np.ascontiguousarray(np.asarray(rel_pos_w, np.float32)[::-1].T).astype(bf16)
    return dict(
        wqk=wqk.astype(bf16), wv=wv, wp=wp, bias=bias, ind=indm.astype(bf16),
        rfh=rfh, rfw=rfw,
    )


def get_nc(reps=1, d_heads=12):
    key = ("nc", reps, d_heads)
    if key not in _CACHE:
        _CACHE[key] = _build_nc(reps=reps, d_heads=d_heads)
    return _CACHE[key]


def make_in_maps(x, qkv_w, rel_pos_h, rel_pos_w, proj_w, proj_b):
    import ml_dtypes

    shared = _host_prep(qkv_w, rel_pos_h, rel_pos_w, proj_w, proj_b)
    x = np.asarray(x, np.float32)
    return [
        dict(shared, xt=np.ascontiguousarray(x[b].T).astype(ml_dtypes.bfloat16))
        for b in range(x.shape[0])
    ]


def kernel(x, qkv_w, rel_pos_h, rel_pos_w, proj_w, proj_b, H=32, W=32):
    from concourse.bass_utils import run_bass_kernel_spmd

    nc = get_nc()
    in_maps = make_in_maps(x, qkv_w, rel_pos_h, rel_pos_w, proj_w, proj_b)
    res = run_bass_kernel_spmd(nc, in_maps, list(range(NCORES)))
    out = np.stack([np.asarray(res.results[b]["y"]) for b in range(NCORES)])
    return out.astype(np.float32)


# revision 11
# speedup vs baseline: 1.0646x; 1.0646x over previous
"""Trainium2 Bass kernel for windowed ViT attention with decomposed relative
position bias (B=8, N=1024=32x32, C=768, 12 heads, head_dim 64).

Sharding: data-parallel over batch B across 8 NeuronCores (1 image per core).

Per-core algorithm (v3):
  - Entire operand path is bf16 (x, qkv/proj weights, rel tables, attention
    operands): FWL on every matmul, half the DMA/SBUF traffic of f32r, and
    f32 PSUM accumulation everywhere.  Adds ~5e-3 relative error
    (budget 2e-2).
  - q/k computed in transposed layout qT/kT [d, n]; q-scale folded into the
    q rows of the qkv weight on the host.
  - rel-pos bias folded into the attention matmul by augmenting the
    contraction dim from 64 to exactly 128:
       S_T[k2, q] = sum_d kT[d,k2] qT[d,q]
                  + sum_i Ih[i,k2] rel_hT[i,q] + sum_j Iw[j,k2] rel_wT[j,q]
    with constant 0/1 indicator rows and Toeplitz-sliced rel tables.
  - rel rows computed 2-h-at-a-time into 2-bank PSUM tiles, evacuated with
    one strided copy per pair, alternating DVE/ACT.
  - All of q/rel/v/k projection runs in phase B/C; the k weights stay
    resident in SBUF (bf16) so there are no per-rep weight DMAs; kasm holds
    all 6 head pairs.
  - Phase D is pure: PE does S/O matmuls, ACT does only exp (fused with
    PSUM evacuation), DVE does reciprocal + normalize straight out of the
    o_ps PSUM tile (no unnorm SBUF copy), gpsimd broadcasts the
    reciprocal row.  softmax denominator rides as a ones-column appended
    to V (attnV out has 65 rows, free on PE).
  - PSUM phase D: s_ps 2x2 banks + o_ps 2x2 banks = 8.
  - phase E (output projection) borrows s_ps PSUM slots; its first 5
    k-blocks per chunk are issued during the denominator drain of the last
    heads; xt for the next rep is prefetched mid-phase-D.
"""

import sys

if "/opt/trn_rl_repo" not in sys.path:
    sys.path.insert(0, "/opt/trn_rl_repo")

import numpy as np

NUM_HEADS = 12
N_CTX = 1024
C_DIM = 768
HD = 64
HH = 32
NCORES = 8

_CACHE: dict = {}


def _build_nc(reps=1, d_heads=12):
    import concourse.mybir as mybir
    import concourse.tile as tile
    from concourse import bacc
    from contextlib import ExitStack

    f32 = mybir.dt.float32
    bf16 = mybir.dt.bfloat16
    f8 = mybir.dt.float8e4
    DR = mybir.MatmulPerfMode.DoubleRow
    Exp = mybir.ActivationFunctionType.Exp

    nc = bacc.Bacc("TRN2", target_bir_lowering=False, debug=False)

    def mm(out, lhsT, rhs, **kw):
        nc.tensor.matmul(out, lhsT, rhs, **kw)

    xt = nc.dram_tensor("xt", [768, 1024], bf16, kind="ExternalInput").ap()
    wqk = nc.dram_tensor("wqk", [768, 1536], bf16, kind="ExternalInput").ap()
    wv = nc.dram_tensor("wv", [768, 768], bf16, kind="ExternalInput").ap()
    wp = nc.dram_tensor("wp", [768, 768], bf16, kind="ExternalInput").ap()
    bias = nc.dram_tensor("bias", [128, 768], f32, kind="ExternalInput").ap()
    ind = nc.dram_tensor("ind", [64, 1024], bf16, kind="ExternalInput").ap()
    rfh = nc.dram_tensor("rfh", [64, 63], bf16, kind="ExternalInput").ap()
    rfw = nc.dram_tensor("rfw", [64, 63], bf16, kind="ExternalInput").ap()
    y = nc.dram_tensor("y", [1024, 768], bf16, kind="ExternalOutput").ap()

    with tile.TileContext(nc) as tc, ExitStack() as es:
        singles = es.enter_context(tc.tile_pool(name="singles", bufs=1))

        # qaug: per head a [128, 1024] aug-rhs block: rows 0:64 = qT (scaled),
        # 64:96 = rel_hT, 96:128 = rel_wT. Heads side by side in columns.
        qaug = singles.tile([64, 2, 12 * 1024], f8)
        # v in natural layout + ones column per head: [k2-part, chunk, head, 65]
        vaug = singles.tile([128, 8, 12, 65], bf16)
        rfh_sb = singles.tile([64, 63], bf16)
        rfw_sb = singles.tile([64, 63], bf16)
        # Assembled S_T lhsT tiles: rows 0:64 = kT chunk, rows 64:128 =
        # constant indicator rows. Axes: [pair, head-parity, chunk].
        kasm = singles.tile([64, 2, 6, 2, 8, 128], f8)
        xt_sb = singles.tile([128, 6, 1024], bf16)
        wq_sb = singles.tile([128, 6, 768], bf16)
        wk_sb = singles.tile([128, 6, 768], bf16)
        wv_sb = singles.tile([128, 6, 768], bf16)
        wp_sb = singles.tile([128, 6, 768], bf16)
        bias_sb = singles.tile([128, 768], f32)
        atile = singles.tile([128, 6, 1024], bf16)  # attn out, [c, n] layout

        xt_r = xt.rearrange("(ko p) n -> p ko n", p=128)
        wqk_r = wqk.rearrange("(ko p) n -> p ko n", p=128)
        wv_r = wv.rearrange("(ko p) n -> p ko n", p=128)
        wp_r = wp.rearrange("(ko p) n -> p ko n", p=128)

        # ---- one-time loads: weights, tables, indicator rows, exp warm ----
        with ExitStack() as es0:
            init = es0.enter_context(tc.tile_pool(name="init", bufs=1))
            ind_sb = init.tile([64, 1024], bf16)
            nc.gpsimd.dma_start(rfh_sb, rfh)
            nc.gpsimd.dma_start(rfw_sb, rfw)
            nc.gpsimd.dma_start(ind_sb, ind)
            for k in range(0, 6, 2):
                nc.sync.dma_start(xt_sb[:, k], xt_r[:, k])
                nc.gpsimd.dma_start(xt_sb[:, k + 1], xt_r[:, k + 1])
            for k in range(0, 6, 2):
                nc.sync.dma_start(wq_sb[:, k], wqk_r[:, k, 0:768])
                nc.gpsimd.dma_start(wq_sb[:, k + 1], wqk_r[:, k + 1, 0:768])
            for k in range(0, 6, 2):
                nc.sync.dma_start(wk_sb[:, k], wqk_r[:, k, 768:1536])
                nc.gpsimd.dma_start(wk_sb[:, k + 1], wqk_r[:, k + 1, 768:1536])
            for k in range(6):
                nc.gpsimd.dma_start(wv_sb[:, k], wv_r[:, k])
                nc.gpsimd.dma_start(wp_sb[:, k], wp_r[:, k])
            nc.gpsimd.dma_start(bias_sb, bias)
            for t in range(6):
                for p in range(2):
                    nc.vector.tensor_copy(
                        kasm[:, 1, t, p],
                        ind_sb.rearrange("p (c n) -> p c n", c=8),
                    )
            nc.vector.memset(vaug[:, :, :, 64:65], 1.0)
            warm = init.tile([1, 1], f32)
            nc.vector.memset(warm, 0.0)
            nc.scalar.activation(warm, warm, Exp)

        for _rep in range(reps):
          with ExitStack() as esR:
            # ------- Phase B/C: q, rel-pos rows, v, k projections -------
            with ExitStack() as esB:
                esQ = esB.enter_context(ExitStack())
                bqk = esQ.enter_context(tc.tile_pool(name="bqk", bufs=2, space="PSUM"))

                # q, transposed layout: out rows = head*64+d, cols = n
                for m in range(6):
                    for n in range(2):
                        ps = bqk.tile([128, 512], f32)
                        for k in range(6):
                            mm(
                                ps,
                                wq_sb[:, k, m * 128 : (m + 1) * 128],
                                xt_sb[:, k, n * 512 : (n + 1) * 512],
                                start=(k == 0),
                                stop=(k == 5),
                            )
                        for half, hd in ((0, 2 * m), (64, 2 * m + 1)):
                            dst = qaug[0:64, 0, hd * 1024 + n * 512 : hd * 1024 + (n + 1) * 512]
                            if m % 2 == 0:
                                nc.scalar.copy(dst, ps[half : half + 64, :])
                            else:
                                nc.vector.tensor_copy(dst, ps[half : half + 64, :])

                # rel-pos rows: rel_hT[i,(head,h,w)] = sum_c rel_pos_h[h-i+31,c]
                # * qT[c,(head,h,w)]; 2 h's per 2-bank PSUM tile, one strided
                # evac copy per pair, alternating DVE/ACT.
                esQ.close()
                bv = esB.enter_context(tc.tile_pool(name="bv", bufs=2, space="PSUM"))
                esRel = esB.enter_context(ExitStack())
                cps = esRel.enter_context(tc.tile_pool(name="cps", bufs=2, space="PSUM"))
                qaug4d = qaug.rearrange("p i (hd a b) -> p i hd a b", hd=12, a=32)
                for hh in range(0, 32, 2):
                    pg = cps.tile([32, 2, 512], f32, tag="cps")
                    pgv = pg[:, :, 0:384].rearrange("p a (c b) -> p a c b", c=12)
                    for i in range(2):
                        mm(pg[:, i, 0:384].rearrange("p (c b) -> p c b", c=12),
                           rfh_sb[:, 31 - hh - i : 63 - hh - i],
                           qaug4d[0:64, 0, :, hh + i, :],
                           start=True, stop=True)
                    dst = qaug4d[0:32, 1, :, hh : hh + 2, :].rearrange(
                        "p c a b -> p a c b")
                    if hh % 4 == 0:
                        nc.vector.tensor_copy(dst, pgv)
                    else:
                        nc.scalar.copy(dst, pgv)
                for ww in range(0, 32, 2):
                    pg = cps.tile([32, 2, 512], f32, tag="cps")
                    pgv = pg[:, :, 0:384].rearrange("p a (c b) -> p a c b", c=12)
                    for j in range(2):
                        mm(pg[:, j, 0:384].rearrange("p (c b) -> p c b", c=12),
                           rfw_sb[:, 31 - ww - j : 63 - ww - j],
                           qaug4d[0:64, 0, :, :, ww + j],
                           start=True, stop=True)
                    dst = qaug4d[32:64, 1, :, :, ww : ww + 2].rearrange(
                        "p c b a -> p a c b")
                    if ww % 4 == 0:
                        nc.vector.tensor_copy(dst, pgv)
                    else:
                        nc.scalar.copy(dst, pgv)

                # v in natural layout [n, c]
                for ch in range(8):
                    pv = bv.tile([128, 768], f32)
                    for c0, cw in ((0, 512), (512, 256)):
                        for k in range(6):
                            mm(
                                pv[:, c0 : c0 + cw],
                                xt_sb[:, k, ch * 128 : (ch + 1) * 128],
                                wv_sb[:, k, c0 : c0 + cw],
                                start=(k == 0),
                                stop=(k == 5),
                            )
                    nc.vector.tensor_copy(
                        vaug[:, ch, :, 0:64], pv.rearrange("p (h d) -> p h d", h=12)
                    )

                # k rows for all 6 pairs -> kasm[t][*][*] rows 0:64
                # (kpp reuses the banks cps frees; rel evacs are done by now)
                esRel.close()
                kpp = esB.enter_context(tc.tile_pool(name="kpp", bufs=4, space="PSUM"))
                for t in range(6):
                    kp = [kpp.tile([128, 512], f32, name=f"kp{n}", tag="kp")
                          for n in range(2)]
                    for n in range(2):
                        for k in range(6):
                            mm(
                                kp[n],
                                wk_sb[:, k, t * 128 : (t + 1) * 128],
                                xt_sb[:, k, n * 512 : (n + 1) * 512],
                                start=(k == 0),
                                stop=(k == 5),
                            )
                        kp4 = kp[n].rearrange("p (c n2) -> p c n2", c=4)
                        for p in range(2):
                            dst = kasm[:, 0, t, p, 4 * n : 4 * n + 4]
                            if n == 0:
                                nc.vector.tensor_copy(dst, kp4[64 * p : 64 * p + 64])
                            else:
                                nc.scalar.copy(dst, kp4[64 * p : 64 * p + 64])

            # ---------------- Phase D: attention per head ----------------
            expp = esR.enter_context(tc.tile_pool(name="expp", bufs=3))
            recp = esR.enter_context(tc.tile_pool(name="recp", bufs=2))
            dps = esR.enter_context(tc.tile_pool(name="dps", bufs=2, space="PSUM"))
            dpo = esR.enter_context(tc.tile_pool(name="dpo", bufs=2, space="PSUM"))
            epool = esR.enter_context(tc.tile_pool(name="epool", bufs=3))

            o_ps_h = {}
            rec_h = {}

            def yblock(ch, klist, stop):
                yp = yp_ch[ch]
                for k in klist:
                    for c0, cw in ((0, 512), (512, 256)):
                        mm(
                            yp[:, c0 : c0 + cw],
                            atile[:, k, ch * 128 : (ch + 1) * 128],
                            wp_sb[:, k, c0 : c0 + cw],
                            start=(k == 0),
                            stop=(stop and k == 5),
                        )

            yp_ch = {}

            for it in range(d_heads + 2):
                # stage A (head=it-1): reciprocal of den row of o_ps;
                # broadcast the reciprocal over 64 partitions (gpsimd)
                if 0 <= it - 1 < d_heads:
                    hd = it - 1
                    o_ps = o_ps_h[hd]
                    rec = recp.tile([1, 2, 512], f32, tag="rc")
                    nc.vector.reciprocal(rec, o_ps[64:65])
                    rep = recp.tile([64, 2, 512], f32, tag="bc")
                    rec_h[hd] = rep
                    nc.gpsimd.partition_broadcast(rep, rec)

                # stage B (head=it-2): normalize straight from PSUM into atile
                if 0 <= it - 2 < d_heads:
                    hd = it - 2
                    t3 = hd // 2
                    half3 = (hd % 2) * 64
                    rep = rec_h.pop(hd)
                    o_ps = o_ps_h.pop(hd)
                    a3 = atile[half3 : half3 + 64, t3, :].rearrange(
                        "p (a b) -> p a b", a=2
                    )
                    nc.vector.tensor_mul(a3, o_ps[0:64], rep)

                # early phase E: k-blocks 0..4 only need heads 0..9, which are
                # normalized by it=11; fills the denominator-drain PE idle.
                # Only 2 chunks fit the 2 "sps" PSUM slots.
                if it == d_heads:
                    for ch in range(2):
                        yp_ch[ch] = dps.tile([128, 768], f32, tag="sps", name="yp")
                        yblock(ch, range(5), stop=False)

                # prefetch next rep's xt once the last phase-B/C consumer is done
                if it == 2 and _rep + 1 < reps:
                    for k in range(0, 6, 2):
                        nc.sync.dma_start(xt_sb[:, k], xt_r[:, k])
                        nc.gpsimd.dma_start(xt_sb[:, k + 1], xt_r[:, k + 1])

                # stage 0 (head=it): attention chunks
                if it < d_heads:
                    hd = it
                    par = hd % 2
                    t = hd // 2
                    o_ps = dpo.tile([65, 2, 512], f32, name="ops", tag="ops")
                    o_ps_h[hd] = o_ps
                    for ch in range(8):
                        s_ps = dps.tile([128, 1024], f32, tag="sps")
                        for nt in range(2):
                            mm(
                                s_ps[:, nt * 512 : (nt + 1) * 512],
                                kasm[:, :, t, par, ch],
                                qaug[:, :, hd * 1024 + nt * 512 : hd * 1024 + (nt + 1) * 512],
                                start=True,
                                stop=True,
                                perf_mode=DR,
                            )
                        ex = expp.tile([128, 1024], bf16)
                        nc.scalar.activation(ex, s_ps, Exp)
                        for nt in range(2):
                            mm(
                                o_ps[:, nt],
                                vaug[:, ch, hd, :],
                                ex[:, nt * 512 : (nt + 1) * 512],
                                start=(ch == 0),
                                stop=(ch == 7),
                            )

            # ---------------- Phase E: output projection (tail) ----------------
            for ch in range(8):
                if ch < 2:
                    yblock(ch, [5], stop=True)
                else:
                    yp_ch[ch] = dps.tile([128, 768], f32, tag="sps", name="yp")
                    yblock(ch, range(6), stop=True)
                yp = yp_ch.pop(ch)
                y_sb = epool.tile([128, 768], bf16)
                nc.vector.tensor_add(y_sb, yp, bias_sb)
                nc.sync.dma_start(y[ch * 128 : (ch + 1) * 128, :], y_sb)

    nc.compile()
    return nc


def _host_prep(qkv_w, rel_pos_h, rel_pos_w, proj_w, proj_b):
    import ml_dtypes

    bf16 = ml_dtypes.bfloat16
    qkv_w = np.asarray(qkv_w, np.float32)
    scale = 1.0 / np.sqrt(HD)
    wqk = np.ascontiguousarray(qkv_w[0:1536].T)  # [768, 1536]
    wqk[:, 0:768] *= scale
    wv = np.ascontiguousarray(qkv_w[1536:2304].T).astype(bf16)  # [768, 768]
    wp = np.ascontiguousarray(np.asarray(proj_w, np.float32).T).astype(bf16)
    bias = np.ascontiguousarray(
        np.broadcast_to(np.asarray(proj_b, np.float32)[None, :], (128, 768))
    )
    k2 = np.arange(1024)
    indm = np.zeros((64, 1024), np.float32)
    indm[0:32] = (k2[None, :] // 32) == np.arange(32)[:, None]
    indm[32:64] = (k2[None, :] % 32) == np.arange(32)[:, None]
    rfh = np.ascontiguousarray(np.asarray(rel_pos_h, np.float32)[::-1].T).astype(bf16)
    rfw = np.ascontiguousarray(np.asarray(rel_pos_w, np.float32)[::-1].T).astype(bf16)
    return dict(
        wqk=wqk.astype(bf16), wv=wv, wp=wp, bias=bias, ind=indm.astype(bf16),
        rfh=rfh, rfw=rfw,
    )


def get_nc(reps=1, d_heads=12):
    key = ("nc", reps, d_heads)
    if key not in _CACHE:
        _CACHE[key] = _build_nc(reps=reps, d_heads=d_heads)
    return _CACHE[key]


def make_in_maps(x, qkv_w, rel_pos_h, rel_pos_w, proj_w, proj_b):
    import ml_dtypes

    shared = _host_prep(qkv_w, rel_pos_h, rel_pos_w, proj_w, proj_b)
    x = np.asarray(x, np.float32)
    return [
        dict(shared, xt=np.ascontiguousarray(x[b].T).astype(ml_dtypes.bfloat16))
        for b in range(x.shape[0])
    ]


def kernel(x, qkv_w, rel_pos_h, rel_pos_w, proj_w, proj_b, H=32, W=32):
    from concourse.bass_utils import run_bass_kernel_spmd

    nc = get_nc()
    in_maps = make_in_maps(x, qkv_w, rel_pos_h, rel_pos_w, proj_w, proj_b)
    res = run_bass_kernel_spmd(nc, in_maps, list(range(NCORES)))
    out = np.stack([np.asarray(res.results[b]["y"]) for b in range(NCORES)])
    return out.astype(np.float32)


# revision 12
# speedup vs baseline: 1.2446x; 1.1690x over previous
"""Trainium2 Bass kernel for windowed ViT attention with decomposed relative
position bias (B=8, N=1024=32x32, C=768, 12 heads, head_dim 64).

Sharding: data-parallel over batch B across 8 NeuronCores (1 image per core).

Per-core algorithm (v3):
  - Entire operand path is bf16 (x, qkv/proj weights, rel tables, attention
    operands): FWL on every matmul, half the DMA/SBUF traffic of f32r, and
    f32 PSUM accumulation everywhere.  Adds ~5e-3 relative error
    (budget 2e-2).
  - q/k computed in transposed layout qT/kT [d, n]; q-scale folded into the
    q rows of the qkv weight on the host.
  - rel-pos bias folded into the attention matmul by augmenting the
    contraction dim from 64 to exactly 128:
       S_T[k2, q] = sum_d kT[d,k2] qT[d,q]
                  + sum_i Ih[i,k2] rel_hT[i,q] + sum_j Iw[j,k2] rel_wT[j,q]
    with constant 0/1 indicator rows and Toeplitz-sliced rel tables.
  - rel rows computed 2-h-at-a-time into 2-bank PSUM tiles, evacuated with
    one strided copy per pair, alternating DVE/ACT.
  - All of q/rel/v/k projection runs in phase B/C; the k weights stay
    resident in SBUF (bf16) so there are no per-rep weight DMAs; kasm holds
    all 6 head pairs.
  - Phase D is pure: PE does S/O matmuls, ACT does only exp (fused with
    PSUM evacuation), DVE does reciprocal + normalize straight out of the
    o_ps PSUM tile (no unnorm SBUF copy), gpsimd broadcasts the
    reciprocal row.  softmax denominator rides as a ones-column appended
    to V (attnV out has 65 rows, free on PE).
  - PSUM phase D: s_ps 2x2 banks + o_ps 2x2 banks = 8.
  - phase E (output projection) borrows s_ps PSUM slots; its first 5
    k-blocks per chunk are issued during the denominator drain of the last
    heads; xt for the next rep is prefetched mid-phase-D.
"""

import sys

if "/opt/trn_rl_repo" not in sys.path:
    sys.path.insert(0, "/opt/trn_rl_repo")

import numpy as np

NUM_HEADS = 12
N_CTX = 1024
C_DIM = 768
HD = 64
HH = 32
NCORES = 8

_CACHE: dict = {}


def _build_nc(reps=1, d_heads=12):
    import concourse.mybir as mybir
    import concourse.tile as tile
    from concourse import bacc
    from contextlib import ExitStack

    f32 = mybir.dt.float32
    bf16 = mybir.dt.bfloat16
    Exp = mybir.ActivationFunctionType.Exp

    nc = bacc.Bacc("TRN2", target_bir_lowering=False, debug=False)

    def mm(out, lhsT, rhs, **kw):
        nc.tensor.matmul(out, lhsT, rhs, **kw)

    xt = nc.dram_tensor("xt", [768, 1024], bf16, kind="ExternalInput").ap()
    wqk = nc.dram_tensor("wqk", [768, 1536], bf16, kind="ExternalInput").ap()
    wv = nc.dram_tensor("wv", [768, 768], bf16, kind="ExternalInput").ap()
    wp = nc.dram_tensor("wp", [768, 768], bf16, kind="ExternalInput").ap()
    bias = nc.dram_tensor("bias", [128, 768], f32, kind="ExternalInput").ap()
    ind = nc.dram_tensor("ind", [64, 1024], bf16, kind="ExternalInput").ap()
    rfh = nc.dram_tensor("rfh", [64, 63], bf16, kind="ExternalInput").ap()
    rfw = nc.dram_tensor("rfw", [64, 63], bf16, kind="ExternalInput").ap()
    y = nc.dram_tensor("y", [1024, 768], bf16, kind="ExternalOutput").ap()

    with tile.TileContext(nc) as tc, ExitStack() as es:
        singles = es.enter_context(tc.tile_pool(name="singles", bufs=1))

        # qaug: per head a [128, 1024] aug-rhs block: rows 0:64 = qT (scaled),
        # 64:96 = rel_hT, 96:128 = rel_wT. Heads side by side in columns.
        qaug = singles.tile([128, 12 * 1024], bf16)
        # v in natural layout + ones column per head: [k2-part, chunk, head, 65]
        vaug = singles.tile([128, 8, 12, 65], bf16)
        rfh_sb = singles.tile([64, 63], bf16)
        rfw_sb = singles.tile([64, 63], bf16)
        # Assembled S_T lhsT tiles: rows 0:64 = kT chunk, rows 64:128 =
        # constant indicator rows. Axes: [pair, head-parity, chunk].
        kasm = singles.tile([128, 6, 2, 8, 128], bf16)
        xt_sb = singles.tile([128, 6, 1024], bf16)
        wq_sb = singles.tile([128, 6, 768], bf16)
        wk_sb = singles.tile([128, 6, 768], bf16)
        wv_sb = singles.tile([128, 6, 768], bf16)
        wp_sb = singles.tile([128, 6, 768], bf16)
        bias_sb = singles.tile([128, 768], f32)
        atile = singles.tile([128, 6, 1024], bf16)  # attn out, [c, n] layout

        xt_r = xt.rearrange("(ko p) n -> p ko n", p=128)
        wqk_r = wqk.rearrange("(ko p) n -> p ko n", p=128)
        wv_r = wv.rearrange("(ko p) n -> p ko n", p=128)
        wp_r = wp.rearrange("(ko p) n -> p ko n", p=128)

        # ---- one-time loads: weights, tables, indicator rows, exp warm ----
        with ExitStack() as es0:
            init = es0.enter_context(tc.tile_pool(name="init", bufs=1))
            ind_sb = init.tile([64, 1024], bf16)
            nc.gpsimd.dma_start(rfh_sb, rfh)
            nc.gpsimd.dma_start(rfw_sb, rfw)
            nc.gpsimd.dma_start(ind_sb, ind)
            for k in range(0, 6, 2):
                nc.sync.dma_start(xt_sb[:, k], xt_r[:, k])
                nc.gpsimd.dma_start(xt_sb[:, k + 1], xt_r[:, k + 1])
            for k in range(0, 6, 2):
                nc.sync.dma_start(wq_sb[:, k], wqk_r[:, k, 0:768])
                nc.gpsimd.dma_start(wq_sb[:, k + 1], wqk_r[:, k + 1, 0:768])
            for k in range(0, 6, 2):
                nc.sync.dma_start(wk_sb[:, k], wqk_r[:, k, 768:1536])
                nc.gpsimd.dma_start(wk_sb[:, k + 1], wqk_r[:, k + 1, 768:1536])
            for k in range(6):
                nc.gpsimd.dma_start(wv_sb[:, k], wv_r[:, k])
                nc.gpsimd.dma_start(wp_sb[:, k], wp_r[:, k])
            nc.gpsimd.dma_start(bias_sb, bias)
            for t in range(6):
                for p in range(2):
                    nc.vector.tensor_copy(
                        kasm[64:128, t, p],
                        ind_sb.rearrange("p (c n) -> p c n", c=8),
                    )
            nc.vector.memset(vaug[:, :, :, 64:65], 1.0)
            warm = init.tile([1, 1], f32)
            nc.vector.memset(warm, 0.0)
            nc.scalar.activation(warm, warm, Exp)

        for _rep in range(reps):
          with ExitStack() as esR:
            # ------- Phase B/C: q, rel-pos rows, v, k projections -------
            with ExitStack() as esB:
                esQ = esB.enter_context(ExitStack())
                bqk = esQ.enter_context(tc.tile_pool(name="bqk", bufs=2, space="PSUM"))

                # q, transposed layout: out rows = head*64+d, cols = n
                for m in range(6):
                    ps = bqk.tile([128, 1024], f32)
                    for n in range(2):
                        for k in range(6):
                            mm(
                                ps[:, n * 512 : (n + 1) * 512],
                                wq_sb[:, k, m * 128 : (m + 1) * 128],
                                xt_sb[:, k, n * 512 : (n + 1) * 512],
                                start=(k == 0),
                                stop=(k == 5),
                            )
                    for half, hd in ((0, 2 * m), (64, 2 * m + 1)):
                        dst = qaug[0:64, hd * 1024 : (hd + 1) * 1024]
                        if m % 2 == 0:
                            nc.scalar.copy(dst, ps[half : half + 64, :])
                        else:
                            nc.vector.tensor_copy(dst, ps[half : half + 64, :])

                # rel-pos rows: rel_hT[i,(head,h,w)] = sum_c rel_pos_h[h-i+31,c]
                # * qT[c,(head,h,w)]; 2 h's per 2-bank PSUM tile, one strided
                # evac copy per pair, alternating DVE/ACT.
                esQ.close()
                bv = esB.enter_context(tc.tile_pool(name="bv", bufs=2, space="PSUM"))
                esRel = esB.enter_context(ExitStack())
                cps = esRel.enter_context(tc.tile_pool(name="cps", bufs=2, space="PSUM"))
                qaug4d = qaug.rearrange("p (hd a b) -> p hd a b", hd=12, a=32)
                for hh in range(0, 32, 2):
                    pg = cps.tile([32, 2, 512], f32, tag="cps")
                    pgv = pg[:, :, 0:384].rearrange("p a (c b) -> p a c b", c=12)
                    for i in range(2):
                        mm(pg[:, i, 0:384].rearrange("p (c b) -> p c b", c=12),
                           rfh_sb[:, 31 - hh - i : 63 - hh - i],
                           qaug4d[0:64, :, hh + i, :],
                           start=True, stop=True)
                    dst = qaug4d[64:96, :, hh : hh + 2, :].rearrange(
                        "p c a b -> p a c b")
                    if hh % 4 == 0:
                        nc.vector.tensor_copy(dst, pgv)
                    else:
                        nc.scalar.copy(dst, pgv)
                for ww in range(0, 32, 2):
                    pg = cps.tile([32, 2, 512], f32, tag="cps")
                    pgv = pg[:, :, 0:384].rearrange("p a (c b) -> p a c b", c=12)
                    for j in range(2):
                        mm(pg[:, j, 0:384].rearrange("p (c b) -> p c b", c=12),
                           rfw_sb[:, 31 - ww - j : 63 - ww - j],
                           qaug4d[0:64, :, :, ww + j],
                           start=True, stop=True)
                    dst = qaug4d[96:128, :, :, ww : ww + 2].rearrange(
                        "p c b a -> p a c b")
                    if ww % 4 == 0:
                        nc.vector.tensor_copy(dst, pgv)
                    else:
                        nc.scalar.copy(dst, pgv)

                # v in natural layout [n, c]
                for ch in range(8):
                    pv = bv.tile([128, 768], f32)
                    for c0, cw in ((0, 512), (512, 256)):
                        for k in range(6):
                            mm(
                                pv[:, c0 : c0 + cw],
                                xt_sb[:, k, ch * 128 : (ch + 1) * 128],
                                wv_sb[:, k, c0 : c0 + cw],
                                start=(k == 0),
                                stop=(k == 5),
                            )
                    nc.vector.tensor_copy(
                        vaug[:, ch, :, 0:64], pv.rearrange("p (h d) -> p h d", h=12)
                    )

                # k rows for all 6 pairs -> kasm[t][*][*] rows 0:64
                # (kpp reuses the banks cps frees; rel evacs are done by now)
                esRel.close()
                kpp = esB.enter_context(tc.tile_pool(name="kpp", bufs=2, space="PSUM"))
                for t in range(6):
                    kp = kpp.tile([128, 1024], f32, name="kp", tag="kp")
                    for n in range(2):
                        for k in range(6):
                            mm(
                                kp[:, n * 512 : (n + 1) * 512],
                                wk_sb[:, k, t * 128 : (t + 1) * 128],
                                xt_sb[:, k, n * 512 : (n + 1) * 512],
                                start=(k == 0),
                                stop=(k == 5),
                            )
                    kp8 = kp.rearrange("p (c n2) -> p c n2", c=8)
                    for p in range(2):
                        dst = kasm[0:64, t, p]
                        if t % 2 == 0:
                            nc.vector.tensor_copy(dst, kp8[64 * p : 64 * p + 64])
                        else:
                            nc.scalar.copy(dst, kp8[64 * p : 64 * p + 64])

            # ---------------- Phase D: attention per head ----------------
            expp = esR.enter_context(tc.tile_pool(name="expp", bufs=3))
            recp = esR.enter_context(tc.tile_pool(name="recp", bufs=2))
            dps = esR.enter_context(tc.tile_pool(name="dps", bufs=2, space="PSUM"))
            dpo = esR.enter_context(tc.tile_pool(name="dpo", bufs=2, space="PSUM"))
            epool = esR.enter_context(tc.tile_pool(name="epool", bufs=3))

            o_ps_h = {}
            rec_h = {}

            def yblock(ch, klist, stop):
                yp = yp_ch[ch]
                for k in klist:
                    for c0, cw in ((0, 512), (512, 256)):
                        mm(
                            yp[:, c0 : c0 + cw],
                            atile[:, k, ch * 128 : (ch + 1) * 128],
                            wp_sb[:, k, c0 : c0 + cw],
                            start=(k == 0),
                            stop=(stop and k == 5),
                        )

            yp_ch = {}

            for it in range(d_heads + 2):
                # stage A (head=it-1): reciprocal of den row of o_ps;
                # broadcast the reciprocal over 64 partitions (gpsimd)
                if 0 <= it - 1 < d_heads:
                    hd = it - 1
                    o_ps = o_ps_h[hd]
                    rec = recp.tile([1, 2, 512], f32, tag="rc")
                    nc.vector.reciprocal(rec, o_ps[64:65])
                    rep = recp.tile([64, 2, 512], f32, tag="bc")
                    rec_h[hd] = rep
                    nc.gpsimd.partition_broadcast(rep, rec)

                # stage B (head=it-2): normalize straight from PSUM into atile
                if 0 <= it - 2 < d_heads:
                    hd = it - 2
                    t3 = hd // 2
                    half3 = (hd % 2) * 64
                    rep = rec_h.pop(hd)
                    o_ps = o_ps_h.pop(hd)
                    a3 = atile[half3 : half3 + 64, t3, :].rearrange(
                        "p (a b) -> p a b", a=2
                    )
                    nc.vector.tensor_mul(a3, o_ps[0:64], rep)

                # early phase E: k-blocks 0..4 only need heads 0..9, which are
                # normalized by it=11; fills the denominator-drain PE idle.
                # Only 2 chunks fit the 2 "sps" PSUM slots.
                if it == d_heads:
                    for ch in range(2):
                        yp_ch[ch] = dps.tile([128, 768], f32, tag="sps", name="yp")
                        yblock(ch, range(5), stop=False)

                # prefetch next rep's xt once the last phase-B/C consumer is done
                if it == 2 and _rep + 1 < reps:
                    nc.sync.dma_start(xt_sb[:, 0:3], xt_r[:, 0:3])
                    nc.gpsimd.dma_start(xt_sb[:, 3:6], xt_r[:, 3:6])

                # stage 0 (head=it): attention chunks
                if it < d_heads:
                    hd = it
                    par = hd % 2
                    t = hd // 2
                    o_ps = dpo.tile([65, 2, 512], f32, name="ops", tag="ops")
                    o_ps_h[hd] = o_ps
                    for ch in range(8):
                        s_ps = dps.tile([128, 1024], f32, tag="sps")
                        for nt in range(2):
                            mm(
                                s_ps[:, nt * 512 : (nt + 1) * 512],
                                kasm[:, t, par, ch],
                                qaug[:, hd * 1024 + nt * 512 : hd * 1024 + (nt + 1) * 512],
                                start=True,
                                stop=True,
                            )
                        ex = expp.tile([128, 1024], bf16)
                        nc.scalar.activation(ex, s_ps, Exp)
                        for nt in range(2):
                            mm(
                                o_ps[:, nt],
                                vaug[:, ch, hd, :],
                                ex[:, nt * 512 : (nt + 1) * 512],
                                start=(ch == 0),
                                stop=(ch == 7),
                            )

            # ---------------- Phase E: output projection (tail) ----------------
            for ch in range(8):
                if ch < 2:
                    yblock(ch, [5], stop=True)
                else:
                    yp_ch[ch] = dps.tile([128, 768], f32, tag="sps", name="yp")
                    yblock(ch, range(6), stop=True)
                yp = yp_ch.pop(ch)
                y_sb = epool.tile([128, 768], bf16)
                nc.vector.tensor_add(y_sb, yp, bias_sb)
                nc.sync.dma_start(y[ch * 128 : (ch + 1) * 128, :], y_sb)

    nc.compile()
    return nc


def _host_prep(qkv_w, rel_pos_h, rel_pos_w, proj_w, proj_b):
    import ml_dtypes

    bf16 = ml_dtypes.bfloat16
    qkv_w = np.asarray(qkv_w, np.float32)
    scale = 1.0 / np.sqrt(HD)
    wqk = np.ascontiguousarray(qkv_w[0:1536].T)  # [768, 1536]
    wqk[:, 0:768] *= scale
    wv = np.ascontiguousarray(qkv_w[1536:2304].T).astype(bf16)  # [768, 768]
    wp = np.ascontiguousarray(np.asarray(proj_w, np.float32).T).astype(bf16)
    bias = np.ascontiguousarray(
        np.broadcast_to(np.asarray(proj_b, np.float32)[None, :], (128, 768))
    )
    k2 = np.arange(1024)
    indm = np.zeros((64, 1024), np.float32)
    indm[0:32] = (k2[None, :] // 32) == np.arange(32)[:, None]
    indm[32:64] = (k2[None, :] % 32) == np.arange(32)[:, None]
    rfh = np.ascontiguousarray(np.asarray(rel_pos_h, np.float32)[::-1].T).astype(bf16)
    rfw = np.ascontiguousarray(np.asarray(rel_pos_w, np.float32)[::-1].T).astype(bf16)
    return dict(
        wqk=wqk.astype(bf16), wv=wv, wp=wp, bias=bias, ind=indm.astype(bf16),
        rfh=rfh, rfw=rfw,
    )


def get_nc(reps=1, d_heads=12):
    key = ("nc", reps, d_heads)
    if key not in _CACHE:
        _CACHE[key] = _build_nc(reps=reps, d_heads=d_heads)
    return _CACHE[key]


def make_in_maps(x, qkv_w, rel_pos_h, rel_pos_w, proj_w, proj_b):
    import ml_dtypes

    shared = _host_prep(qkv_w, rel_pos_h, rel_pos_w, proj_w, proj_b)
    x = np.asarray(x, np.float32)
    return [
        dict(shared, xt=np.ascontiguousarray(x[b].T).astype(ml_dtypes.bfloat16))
        for b in range(x.shape[0])
    ]


def kernel(x, qkv_w, rel_pos_h, rel_pos_w, proj_w, proj_b, H=32, W=32):
    from concourse.bass_utils import run_bass_kernel_spmd

    nc = get_nc()
    in_maps = make_in_maps(x, qkv_w, rel_pos_h, rel_pos_w, proj_w, proj_b)
    res = run_bass_kernel_spmd(nc, in_maps, list(range(NCORES)))
    out = np.stack([np.asarray(res.results[b]["y"]) for b in range(NCORES)])
    return out.astype(np.float32)


# revision 13
# speedup vs baseline: 1.3163x; 1.0577x over previous
"""Trainium2 Bass kernel for windowed ViT attention with decomposed relative
position bias (B=8, N=1024=32x32, C=768, 12 heads, head_dim 64).

Sharding: data-parallel over batch B across 8 NeuronCores (1 image per core).

Per-core algorithm (v3):
  - Entire operand path is bf16 (x, qkv/proj weights, rel tables, attention
    operands): FWL on every matmul, half the DMA/SBUF traffic of f32r, and
    f32 PSUM accumulation everywhere.  Adds ~5e-3 relative error
    (budget 2e-2).
  - q/k computed in transposed layout qT/kT [d, n]; q-scale folded into the
    q rows of the qkv weight on the host.
  - rel-pos bias folded into the attention matmul by augmenting the
    contraction dim from 64 to exactly 128:
       S_T[k2, q] = sum_d kT[d,k2] qT[d,q]
                  + sum_i Ih[i,k2] rel_hT[i,q] + sum_j Iw[j,k2] rel_wT[j,q]
    with constant 0/1 indicator rows and Toeplitz-sliced rel tables.
  - rel rows computed 2-h-at-a-time into 2-bank PSUM tiles, evacuated with
    one strided copy per pair, alternating DVE/ACT.
  - All of q/rel/v/k projection runs in phase B/C; the k weights stay
    resident in SBUF (bf16) so there are no per-rep weight DMAs; kasm holds
    all 6 head pairs.
  - Phase D is pure: PE does S/O matmuls, ACT does only exp (fused with
    PSUM evacuation), DVE does reciprocal + normalize straight out of the
    o_ps PSUM tile (no unnorm SBUF copy), gpsimd broadcasts the
    reciprocal row.  softmax denominator rides as a ones-column appended
    to V (attnV out has 65 rows, free on PE).
  - PSUM phase D: s_ps 2x2 banks + o_ps 2x2 banks = 8.
  - phase E (output projection) borrows s_ps PSUM slots; its first 5
    k-blocks per chunk are issued during the denominator drain of the last
    heads; xt for the next rep is prefetched mid-phase-D.
"""

import sys

if "/opt/trn_rl_repo" not in sys.path:
    sys.path.insert(0, "/opt/trn_rl_repo")

import numpy as np

NUM_HEADS = 12
N_CTX = 1024
C_DIM = 768
HD = 64
HH = 32
NCORES = 8

_CACHE: dict = {}


def _build_nc(reps=1, d_heads=12):
    import concourse.mybir as mybir
    import concourse.tile as tile
    from concourse import bacc
    from contextlib import ExitStack

    f32 = mybir.dt.float32
    bf16 = mybir.dt.bfloat16
    Exp = mybir.ActivationFunctionType.Exp

    nc = bacc.Bacc("TRN2", target_bir_lowering=False, debug=False)

    def mm(out, lhsT, rhs, **kw):
        nc.tensor.matmul(out, lhsT, rhs, **kw)

    xt = nc.dram_tensor("xt", [768, 1024], bf16, kind="ExternalInput").ap()
    wqk = nc.dram_tensor("wqk", [768, 1536], bf16, kind="ExternalInput").ap()
    wv = nc.dram_tensor("wv", [768, 768], bf16, kind="ExternalInput").ap()
    wp = nc.dram_tensor("wp", [768, 768], bf16, kind="ExternalInput").ap()
    bias = nc.dram_tensor("bias", [128, 768], f32, kind="ExternalInput").ap()
    ind = nc.dram_tensor("ind", [64, 1024], bf16, kind="ExternalInput").ap()
    rfh = nc.dram_tensor("rfh", [64, 63], bf16, kind="ExternalInput").ap()
    rfw = nc.dram_tensor("rfw", [64, 63], bf16, kind="ExternalInput").ap()
    y = nc.dram_tensor("y", [1024, 768], bf16, kind="ExternalOutput").ap()

    with tile.TileContext(nc) as tc, ExitStack() as es:
        singles = es.enter_context(tc.tile_pool(name="singles", bufs=1))

        # qaug: per head a [128, 1024] aug-rhs block: rows 0:64 = qT (scaled),
        # 64:96 = rel_hT, 96:128 = rel_wT. Heads side by side in columns.
        qaug = singles.tile([128, 12 * 1024], bf16)
        # v in natural layout + ones column per head: [k2-part, chunk, head, 65]
        vaug = singles.tile([128, 8, 12, 65], bf16)
        rfh_sb = singles.tile([64, 63], bf16)
        rfw_sb = singles.tile([64, 63], bf16)
        # Assembled S_T lhsT tiles: rows 0:64 = kT chunk, rows 64:128 =
        # constant indicator rows. Axes: [pair, head-parity, chunk].
        kasm = singles.tile([128, 6, 2, 8, 128], bf16)
        xt_sb = singles.tile([128, 6, 1024], bf16)
        wq_sb = singles.tile([128, 6, 768], bf16)
        wk_sb = singles.tile([128, 6, 768], bf16)
        wv_sb = singles.tile([128, 6, 768], bf16)
        wp_sb = singles.tile([128, 6, 768], bf16)
        bias_sb = singles.tile([128, 768], f32)
        atile = singles.tile([128, 6, 1024], bf16)  # attn out, [c, n] layout

        xt_r = xt.rearrange("(ko p) n -> p ko n", p=128)
        wqk_r = wqk.rearrange("(ko p) n -> p ko n", p=128)
        wv_r = wv.rearrange("(ko p) n -> p ko n", p=128)
        wp_r = wp.rearrange("(ko p) n -> p ko n", p=128)

        # ---- one-time loads: weights, tables, indicator rows, exp warm ----
        with ExitStack() as es0:
            init = es0.enter_context(tc.tile_pool(name="init", bufs=1))
            ind_sb = init.tile([64, 1024], bf16)
            nc.gpsimd.dma_start(rfh_sb, rfh)
            nc.gpsimd.dma_start(rfw_sb, rfw)
            nc.gpsimd.dma_start(ind_sb, ind)
            for k in range(0, 6, 2):
                nc.sync.dma_start(xt_sb[:, k], xt_r[:, k])
                nc.gpsimd.dma_start(xt_sb[:, k + 1], xt_r[:, k + 1])
            for k in range(0, 6, 2):
                nc.sync.dma_start(wq_sb[:, k], wqk_r[:, k, 0:768])
                nc.gpsimd.dma_start(wq_sb[:, k + 1], wqk_r[:, k + 1, 0:768])
            for k in range(0, 6, 2):
                nc.sync.dma_start(wk_sb[:, k], wqk_r[:, k, 768:1536])
                nc.gpsimd.dma_start(wk_sb[:, k + 1], wqk_r[:, k + 1, 768:1536])
            for k in range(6):
                nc.gpsimd.dma_start(wv_sb[:, k], wv_r[:, k])
                nc.gpsimd.dma_start(wp_sb[:, k], wp_r[:, k])
            nc.gpsimd.dma_start(bias_sb, bias)
            for t in range(6):
                for p in range(2):
                    nc.vector.tensor_copy(
                        kasm[64:128, t, p],
                        ind_sb.rearrange("p (c n) -> p c n", c=8),
                    )
            nc.vector.memset(vaug[:, :, :, 64:65], 1.0)
            warm = init.tile([1, 1], f32)
            nc.vector.memset(warm, 0.0)
            nc.scalar.activation(warm, warm, Exp)

        for _rep in range(reps):
          with ExitStack() as esR:
            # ------- Phase B/C: q, rel-pos rows, v, k projections -------
            with ExitStack() as esB:
                esQ = esB.enter_context(ExitStack())
                bqk = esQ.enter_context(tc.tile_pool(name="bqk", bufs=2, space="PSUM"))

                # q, transposed layout: out rows = head*64+d, cols = n
                for m in range(6):
                    ps = bqk.tile([128, 1024], f32)
                    for n in range(2):
                        for k in range(6):
                            mm(
                                ps[:, n * 512 : (n + 1) * 512],
                                wq_sb[:, k, m * 128 : (m + 1) * 128],
                                xt_sb[:, k, n * 512 : (n + 1) * 512],
                                start=(k == 0),
                                stop=(k == 5),
                            )
                    for half, hd in ((0, 2 * m), (64, 2 * m + 1)):
                        dst = qaug[0:64, hd * 1024 : (hd + 1) * 1024]
                        if m % 2 == 0:
                            nc.scalar.copy(dst, ps[half : half + 64, :])
                        else:
                            nc.vector.tensor_copy(dst, ps[half : half + 64, :])

                # rel-pos rows: rel_hT[i,(head,h,w)] = sum_c rel_pos_h[h-i+31,c]
                # * qT[c,(head,h,w)]; 2 h's per 2-bank PSUM tile, one strided
                # evac copy per pair, alternating DVE/ACT.
                esQ.close()
                bv = esB.enter_context(tc.tile_pool(name="bv", bufs=2, space="PSUM"))
                esRel = esB.enter_context(ExitStack())
                cps = esRel.enter_context(tc.tile_pool(name="cps", bufs=2, space="PSUM"))
                qaug4d = qaug.rearrange("p (hd a b) -> p hd a b", hd=12, a=32)
                for hh in range(0, 32, 2):
                    pg = cps.tile([32, 2, 512], f32, tag="cps")
                    pgv = pg[:, :, 0:384].rearrange("p a (c b) -> p a c b", c=12)
                    for i in range(2):
                        mm(pg[:, i, 0:384].rearrange("p (c b) -> p c b", c=12),
                           rfh_sb[:, 31 - hh - i : 63 - hh - i],
                           qaug4d[0:64, :, hh + i, :],
                           start=True, stop=True)
                    dst = qaug4d[64:96, :, hh : hh + 2, :].rearrange(
                        "p c a b -> p a c b")
                    if hh % 4 == 0:
                        nc.vector.tensor_copy(dst, pgv)
                    else:
                        nc.scalar.copy(dst, pgv)
                for ww in range(0, 32, 2):
                    pg = cps.tile([32, 2, 512], f32, tag="cps")
                    pgv = pg[:, :, 0:384].rearrange("p a (c b) -> p a c b", c=12)
                    for j in range(2):
                        mm(pg[:, j, 0:384].rearrange("p (c b) -> p c b", c=12),
                           rfw_sb[:, 31 - ww - j : 63 - ww - j],
                           qaug4d[0:64, :, :, ww + j],
                           start=True, stop=True)
                    dst = qaug4d[96:128, :, :, ww : ww + 2].rearrange(
                        "p c b a -> p a c b")
                    if ww % 4 == 0:
                        nc.vector.tensor_copy(dst, pgv)
                    else:
                        nc.scalar.copy(dst, pgv)

                # v in natural layout [n, c]
                for ch in range(8):
                    pv = bv.tile([128, 768], f32)
                    for c0, cw in ((0, 512), (512, 256)):
                        for k in range(6):
                            mm(
                                pv[:, c0 : c0 + cw],
                                xt_sb[:, k, ch * 128 : (ch + 1) * 128],
                                wv_sb[:, k, c0 : c0 + cw],
                                start=(k == 0),
                                stop=(k == 5),
                            )
                    nc.vector.tensor_copy(
                        vaug[:, ch, :, 0:64], pv.rearrange("p (h d) -> p h d", h=12)
                    )

                # k rows for all 6 pairs -> kasm[t][*][*] rows 0:64
                # (kpp reuses the banks cps frees; rel evacs are done by now)
                esRel.close()
                kpp = esB.enter_context(tc.tile_pool(name="kpp", bufs=2, space="PSUM"))
                for t in range(6):
                    kp = kpp.tile([128, 1024], f32, name="kp", tag="kp")
                    for n in range(2):
                        for k in range(6):
                            mm(
                                kp[:, n * 512 : (n + 1) * 512],
                                wk_sb[:, k, t * 128 : (t + 1) * 128],
                                xt_sb[:, k, n * 512 : (n + 1) * 512],
                                start=(k == 0),
                                stop=(k == 5),
                            )
                    kp8 = kp.rearrange("p (c n2) -> p c n2", c=8)
                    for p in range(2):
                        dst = kasm[0:64, t, p]
                        if t % 2 == 0:
                            nc.vector.tensor_copy(dst, kp8[64 * p : 64 * p + 64])
                        else:
                            nc.scalar.copy(dst, kp8[64 * p : 64 * p + 64])

            # ---------------- Phase D: attention per head ----------------
            expp = esR.enter_context(tc.tile_pool(name="expp", bufs=4))
            recp = esR.enter_context(tc.tile_pool(name="recp", bufs=2))
            dps = esR.enter_context(tc.tile_pool(name="dps", bufs=2, space="PSUM"))
            dpo = esR.enter_context(tc.tile_pool(name="dpo", bufs=2, space="PSUM"))
            epool = esR.enter_context(tc.tile_pool(name="epool", bufs=3))

            o_ps_h = {}
            rec_h = {}

            def yblock(ch, klist, stop):
                yp = yp_ch[ch]
                for k in klist:
                    for c0, cw in ((0, 512), (512, 256)):
                        mm(
                            yp[:, c0 : c0 + cw],
                            atile[:, k, ch * 128 : (ch + 1) * 128],
                            wp_sb[:, k, c0 : c0 + cw],
                            start=(k == 0),
                            stop=(stop and k == 5),
                        )

            yp_ch = {}

            for it in range(d_heads + 2):
                # stage A (head=it-1): reciprocal of den row of o_ps;
                # broadcast the reciprocal over 64 partitions (gpsimd)
                if 0 <= it - 1 < d_heads:
                    hd = it - 1
                    o_ps = o_ps_h[hd]
                    rec = recp.tile([1, 2, 512], f32, tag="rc")
                    nc.vector.reciprocal(rec, o_ps[64:65])
                    rep = recp.tile([64, 2, 512], f32, tag="bc")
                    rec_h[hd] = rep
                    nc.gpsimd.partition_broadcast(rep, rec)

                # stage B (head=it-2): normalize straight from PSUM into atile
                if 0 <= it - 2 < d_heads:
                    hd = it - 2
                    t3 = hd // 2
                    half3 = (hd % 2) * 64
                    rep = rec_h.pop(hd)
                    o_ps = o_ps_h.pop(hd)
                    a3 = atile[half3 : half3 + 64, t3, :].rearrange(
                        "p (a b) -> p a b", a=2
                    )
                    nc.vector.tensor_mul(a3, o_ps[0:64], rep)

                # early phase E: k-blocks 0..4 only need heads 0..9, which are
                # normalized by it=11; fills the denominator-drain PE idle.
                # Only 2 chunks fit the 2 "sps" PSUM slots.
                if it == d_heads:
                    for ch in range(2):
                        yp_ch[ch] = dps.tile([128, 768], f32, tag="sps", name="yp")
                        yblock(ch, range(5), stop=False)

                # prefetch next rep's xt once the last phase-B/C consumer is done
                if it == 2 and _rep + 1 < reps:
                    nc.sync.dma_start(xt_sb[:, 0:3], xt_r[:, 0:3])
                    nc.gpsimd.dma_start(xt_sb[:, 3:6], xt_r[:, 3:6])

                # stage 0 (head=it): attention chunks
                if it < d_heads:
                    hd = it
                    par = hd % 2
                    t = hd // 2
                    o_ps = dpo.tile([65, 2, 512], f32, name="ops", tag="ops")
                    o_ps_h[hd] = o_ps

                    def o_emit(pair):
                        ch, ex = pair
                        for nt in range(2):
                            mm(
                                o_ps[:, nt],
                                vaug[:, ch, hd, :],
                                ex[:, nt * 512 : (nt + 1) * 512],
                                start=(ch == 0),
                                stop=(ch == 7),
                            )

                    # O matmuls trail the exp stream by 2 chunks so neither
                    # PE nor ACT ever waits on a fresh cross-engine handoff
                    o_q = []
                    for ch in range(8):
                        s_ps = dps.tile([128, 1024], f32, tag="sps")
                        for nt in range(2):
                            mm(
                                s_ps[:, nt * 512 : (nt + 1) * 512],
                                kasm[:, t, par, ch],
                                qaug[:, hd * 1024 + nt * 512 : hd * 1024 + (nt + 1) * 512],
                                start=True,
                                stop=True,
                            )
                        ex = expp.tile([128, 1024], bf16)
                        nc.scalar.activation(ex, s_ps, Exp)
                        o_q.append((ch, ex))
                        if len(o_q) > 2:
                            o_emit(o_q.pop(0))
                    for pair in o_q:
                        o_emit(pair)

            # ---------------- Phase E: output projection (tail) ----------------
            for ch in range(8):
                if ch < 2:
                    yblock(ch, [5], stop=True)
                else:
                    yp_ch[ch] = dps.tile([128, 768], f32, tag="sps", name="yp")
                    yblock(ch, range(6), stop=True)
                yp = yp_ch.pop(ch)
                y_sb = epool.tile([128, 768], bf16)
                nc.vector.tensor_add(y_sb, yp, bias_sb)
                nc.sync.dma_start(y[ch * 128 : (ch + 1) * 128, :], y_sb)

    nc.compile()
    return nc


def _host_prep(qkv_w, rel_pos_h, rel_pos_w, proj_w, proj_b):
    import ml_dtypes

    bf16 = ml_dtypes.bfloat16
    qkv_w = np.asarray(qkv_w, np.float32)
    scale = 1.0 / np.sqrt(HD)
    wqk = np.ascontiguousarray(qkv_w[0:1536].T)  # [768, 1536]
    wqk[:, 0:768] *= scale
    wv = np.ascontiguousarray(qkv_w[1536:2304].T).astype(bf16)  # [768, 768]
    wp = np.ascontiguousarray(np.asarray(proj_w, np.float32).T).astype(bf16)
    bias = np.ascontiguousarray(
        np.broadcast_to(np.asarray(proj_b, np.float32)[None, :], (128, 768))
    )
    k2 = np.arange(1024)
    indm = np.zeros((64, 1024), np.float32)
    indm[0:32] = (k2[None, :] // 32) == np.arange(32)[:, None]
    indm[32:64] = (k2[None, :] % 32) == np.arange(32)[:, None]
    rfh = np.ascontiguousarray(np.asarray(rel_pos_h, np.float32)[::-1].T).astype(bf16)
    rfw = np.ascontiguousarray(np.asarray(rel_pos_w, np.float32)[::-1].T).astype(bf16)
    return dict(
        wqk=wqk.astype(bf16), wv=wv, wp=wp, bias=bias, ind=indm.astype(bf16),
        rfh=rfh, rfw=rfw,
    )


def get_nc(reps=1, d_heads=12):
    key = ("nc", reps, d_heads)
    if key not in _CACHE:
        _CACHE[key] = _build_nc(reps=reps, d_heads=d_heads)
    return _CACHE[key]


def make_in_maps(x, qkv_w, rel_pos_h, rel_pos_w, proj_w, proj_b):
    import ml_dtypes

    shared = _host_prep(qkv_w, rel_pos_h, rel_pos_w, proj_w, proj_b)
    x = np.asarray(x, np.float32)
    return [
        dict(shared, xt=np.ascontiguousarray(x[b].T).astype(ml_dtypes.bfloat16))
        for b in range(x.shape[0])
    ]


def kernel(x, qkv_w, rel_pos_h, rel_pos_w, proj_w, proj_b, H=32, W=32):
    from concourse.bass_utils import run_bass_kernel_spmd

    nc = get_nc()
    in_maps = make_in_maps(x, qkv_w, rel_pos_h, rel_pos_w, proj_w, proj_b)
    res = run_bass_kernel_spmd(nc, in_maps, list(range(NCORES)))
    out = np.stack([np.asarray(res.results[b]["y"]) for b in range(NCORES)])
    return out.astype(np.float32)


# revision 14
# speedup vs baseline: 1.3539x; 1.0285x over previous
"""Trainium2 Bass kernel for windowed ViT attention with decomposed relative
position bias (B=8, N=1024=32x32, C=768, 12 heads, head_dim 64).

Sharding: data-parallel over batch B across 8 NeuronCores (1 image per core).

Per-core algorithm (v3):
  - Entire operand path is bf16 (x, qkv/proj weights, rel tables, attention
    operands): FWL on every matmul, half the DMA/SBUF traffic of f32r, and
    f32 PSUM accumulation everywhere.  Adds ~5e-3 relative error
    (budget 2e-2).
  - q/k computed in transposed layout qT/kT [d, n]; q-scale folded into the
    q rows of the qkv weight on the host.
  - rel-pos bias folded into the attention matmul by augmenting the
    contraction dim from 64 to exactly 128:
       S_T[k2, q] = sum_d kT[d,k2] qT[d,q]
                  + sum_i Ih[i,k2] rel_hT[i,q] + sum_j Iw[j,k2] rel_wT[j,q]
    with constant 0/1 indicator rows and Toeplitz-sliced rel tables.
  - rel rows computed 2-h-at-a-time into 2-bank PSUM tiles, evacuated with
    one strided copy per pair, alternating DVE/ACT.
  - All of q/rel/v/k projection runs in phase B/C; the k weights stay
    resident in SBUF (bf16) so there are no per-rep weight DMAs; kasm holds
    all 6 head pairs.
  - Phase D is pure: PE does S/O matmuls, ACT does only exp (fused with
    PSUM evacuation), DVE does reciprocal + normalize straight out of the
    o_ps PSUM tile (no unnorm SBUF copy), gpsimd broadcasts the
    reciprocal row.  softmax denominator rides as a ones-column appended
    to V (attnV out has 65 rows, free on PE).
  - PSUM phase D: s_ps 2x2 banks + o_ps 2x2 banks = 8.
  - phase E (output projection) borrows s_ps PSUM slots; its first 5
    k-blocks per chunk are issued during the denominator drain of the last
    heads; xt for the next rep is prefetched mid-phase-D.
"""

import sys

if "/opt/trn_rl_repo" not in sys.path:
    sys.path.insert(0, "/opt/trn_rl_repo")

import numpy as np

NUM_HEADS = 12
N_CTX = 1024
C_DIM = 768
HD = 64
HH = 32
NCORES = 8

_CACHE: dict = {}


def _build_nc(reps=1, d_heads=12):
    import concourse.mybir as mybir
    import concourse.tile as tile
    from concourse import bacc
    from contextlib import ExitStack

    f32 = mybir.dt.float32
    bf16 = mybir.dt.bfloat16
    Exp = mybir.ActivationFunctionType.Exp

    nc = bacc.Bacc("TRN2", target_bir_lowering=False, debug=False)

    def mm(out, lhsT, rhs, **kw):
        nc.tensor.matmul(out, lhsT, rhs, **kw)

    xt = nc.dram_tensor("xt", [768, 1024], bf16, kind="ExternalInput").ap()
    wqk = nc.dram_tensor("wqk", [768, 1536], bf16, kind="ExternalInput").ap()
    wv = nc.dram_tensor("wv", [768, 768], bf16, kind="ExternalInput").ap()
    wp = nc.dram_tensor("wp", [768, 768], bf16, kind="ExternalInput").ap()
    bias = nc.dram_tensor("bias", [128, 768], f32, kind="ExternalInput").ap()
    ind = nc.dram_tensor("ind", [64, 1024], bf16, kind="ExternalInput").ap()
    rfh = nc.dram_tensor("rfh", [64, 63], bf16, kind="ExternalInput").ap()
    rfw = nc.dram_tensor("rfw", [64, 63], bf16, kind="ExternalInput").ap()
    y = nc.dram_tensor("y", [1024, 768], bf16, kind="ExternalOutput").ap()

    with tile.TileContext(nc) as tc, ExitStack() as es:
        singles = es.enter_context(tc.tile_pool(name="singles", bufs=1))

        # qaug: per head a [128, 1024] aug-rhs block: rows 0:64 = qT (scaled),
        # 64:96 = rel_hT, 96:128 = rel_wT. Heads side by side in columns.
        qaug = singles.tile([128, 12 * 1024], bf16)
        # v in natural layout + ones column per head: [k2-part, chunk, head, 65]
        vaug = singles.tile([128, 8, 12, 65], bf16)
        rfh_sb = singles.tile([64, 63], bf16)
        rfw_sb = singles.tile([64, 63], bf16)
        # Assembled S_T lhsT tiles: rows 0:64 = kT chunk, rows 64:128 =
        # constant indicator rows. Axes: [pair, head-parity, chunk].
        kasm = singles.tile([128, 6, 2, 8, 128], bf16)
        xt_sb = singles.tile([128, 6, 1024], bf16)
        wq_sb = singles.tile([128, 6, 768], bf16)
        wk_sb = singles.tile([128, 6, 768], bf16)
        wv_sb = singles.tile([128, 6, 768], bf16)
        wp_sb = singles.tile([128, 6, 768], bf16)
        bias_sb = singles.tile([128, 768], f32)
        atile = singles.tile([128, 6, 1024], bf16)  # attn out, [c, n] layout

        xt_r = xt.rearrange("(ko p) n -> p ko n", p=128)
        wqk_r = wqk.rearrange("(ko p) n -> p ko n", p=128)
        wv_r = wv.rearrange("(ko p) n -> p ko n", p=128)
        wp_r = wp.rearrange("(ko p) n -> p ko n", p=128)

        # ---- one-time loads: weights, tables, indicator rows, exp warm ----
        with ExitStack() as es0:
            init = es0.enter_context(tc.tile_pool(name="init", bufs=1))
            ind_sb = init.tile([64, 1024], bf16)
            nc.gpsimd.dma_start(rfh_sb, rfh)
            nc.gpsimd.dma_start(rfw_sb, rfw)
            nc.gpsimd.dma_start(ind_sb, ind)
            for k in range(0, 6, 2):
                nc.sync.dma_start(xt_sb[:, k], xt_r[:, k])
                nc.gpsimd.dma_start(xt_sb[:, k + 1], xt_r[:, k + 1])
            for k in range(0, 6, 2):
                nc.sync.dma_start(wq_sb[:, k], wqk_r[:, k, 0:768])
                nc.gpsimd.dma_start(wq_sb[:, k + 1], wqk_r[:, k + 1, 0:768])
            for k in range(0, 6, 2):
                nc.sync.dma_start(wk_sb[:, k], wqk_r[:, k, 768:1536])
                nc.gpsimd.dma_start(wk_sb[:, k + 1], wqk_r[:, k + 1, 768:1536])
            for k in range(6):
                nc.gpsimd.dma_start(wv_sb[:, k], wv_r[:, k])
                nc.gpsimd.dma_start(wp_sb[:, k], wp_r[:, k])
            nc.gpsimd.dma_start(bias_sb, bias)
            for t in range(6):
                for p in range(2):
                    nc.vector.tensor_copy(
                        kasm[64:128, t, p],
                        ind_sb.rearrange("p (c n) -> p c n", c=8),
                    )
            nc.vector.memset(vaug[:, :, :, 64:65], 1.0)
            warm = init.tile([1, 1], f32)
            nc.vector.memset(warm, 0.0)
            nc.scalar.activation(warm, warm, Exp)

        for _rep in range(reps):
          with ExitStack() as esR:
            # ------- Phase B/C: q, rel-pos rows, v, k projections -------
            with ExitStack() as esB:
                esQ = esB.enter_context(ExitStack())
                bqk = esQ.enter_context(tc.tile_pool(name="bqk", bufs=2, space="PSUM"))

                # q, transposed layout: out rows = head*64+d, cols = n
                for m in range(6):
                    ps = bqk.tile([128, 1024], f32)
                    for n in range(2):
                        for k in range(6):
                            mm(
                                ps[:, n * 512 : (n + 1) * 512],
                                wq_sb[:, k, m * 128 : (m + 1) * 128],
                                xt_sb[:, k, n * 512 : (n + 1) * 512],
                                start=(k == 0),
                                stop=(k == 5),
                            )
                    for half, hd in ((0, 2 * m), (64, 2 * m + 1)):
                        dst = qaug[0:64, hd * 1024 : (hd + 1) * 1024]
                        if m % 2 == 0:
                            nc.scalar.copy(dst, ps[half : half + 64, :])
                        else:
                            nc.vector.tensor_copy(dst, ps[half : half + 64, :])

                # rel-pos rows: rel_hT[i,(head,h,w)] = sum_c rel_pos_h[h-i+31,c]
                # * qT[c,(head,h,w)]; 2 h's per 2-bank PSUM tile, one strided
                # evac copy per pair, alternating DVE/ACT.
                esQ.close()
                bv = esB.enter_context(tc.tile_pool(name="bv", bufs=2, space="PSUM"))
                esRel = esB.enter_context(ExitStack())
                cps = esRel.enter_context(tc.tile_pool(name="cps", bufs=2, space="PSUM"))
                qaug4d = qaug.rearrange("p (hd a b) -> p hd a b", hd=12, a=32)
                def rel_tile(axis, i0):
                    pg = cps.tile([32, 2, 512], f32, tag="cps", name="pg")
                    pgv = pg[:, :, 0:384].rearrange("p a (c b) -> p a c b", c=12)
                    for i in range(2):
                        if axis == 0:
                            mm(pg[:, i, 0:384].rearrange("p (c b) -> p c b", c=12),
                               rfh_sb[:, 31 - i0 - i : 63 - i0 - i],
                               qaug4d[0:64, :, i0 + i, :],
                               start=True, stop=True)
                        else:
                            mm(pg[:, i, 0:384].rearrange("p (c b) -> p c b", c=12),
                               rfw_sb[:, 31 - i0 - i : 63 - i0 - i],
                               qaug4d[0:64, :, :, i0 + i],
                               start=True, stop=True)
                    if axis == 0:
                        dst = qaug4d[64:96, :, i0 : i0 + 2, :].rearrange(
                            "p c a b -> p a c b")
                    else:
                        dst = qaug4d[96:128, :, :, i0 : i0 + 2].rearrange(
                            "p c b a -> p a c b")
                    if i0 % 4 == 0:
                        nc.vector.tensor_copy(dst, pgv)
                    else:
                        nc.scalar.copy(dst, pgv)

                def v_block(ch):
                    pv = bv.tile([128, 768], f32, name="pv")
                    for c0, cw in ((0, 512), (512, 256)):
                        for k in range(6):
                            mm(
                                pv[:, c0 : c0 + cw],
                                xt_sb[:, k, ch * 128 : (ch + 1) * 128],
                                wv_sb[:, k, c0 : c0 + cw],
                                start=(k == 0),
                                stop=(k == 5),
                            )
                    nc.vector.tensor_copy(
                        vaug[:, ch, :, 0:64], pv.rearrange("p (h d) -> p h d", h=12)
                    )

                # rel tiles interleaved with v blocks: v matmuls fill PE
                # while the rel evacs drain
                vi = 0
                for idx in range(16):
                    axis, i0 = (0, 2 * idx) if idx < 8 else (1, 2 * (idx - 8))
                    rel_tile(axis, i0)
                    if idx % 2 == 1 and vi < 8:
                        v_block(vi)
                        vi += 1
                for idx in range(16, 32):
                    axis, i0 = (0, 2 * (idx - 8)) if idx < 24 else (1, 2 * (idx - 16))
                    rel_tile(axis, i0)
                while vi < 8:
                    v_block(vi)
                    vi += 1

                # k rows for all 6 pairs -> kasm[t][*][*] rows 0:64
                # (kpp reuses the banks cps frees; rel evacs are done by now)
                esRel.close()
                kpp = esB.enter_context(tc.tile_pool(name="kpp", bufs=2, space="PSUM"))
                for t in range(6):
                    kp = kpp.tile([128, 1024], f32, name="kp", tag="kp")
                    for n in range(2):
                        for k in range(6):
                            mm(
                                kp[:, n * 512 : (n + 1) * 512],
                                wk_sb[:, k, t * 128 : (t + 1) * 128],
                                xt_sb[:, k, n * 512 : (n + 1) * 512],
                                start=(k == 0),
                                stop=(k == 5),
                            )
                    kp8 = kp.rearrange("p (c n2) -> p c n2", c=8)
                    for p in range(2):
                        dst = kasm[0:64, t, p]
                        if t % 2 == 0:
                            nc.vector.tensor_copy(dst, kp8[64 * p : 64 * p + 64])
                        else:
                            nc.scalar.copy(dst, kp8[64 * p : 64 * p + 64])

            # ---------------- Phase D: attention per head ----------------
            expp = esR.enter_context(tc.tile_pool(name="expp", bufs=4))
            recp = esR.enter_context(tc.tile_pool(name="recp", bufs=2))
            dps = esR.enter_context(tc.tile_pool(name="dps", bufs=2, space="PSUM"))
            dpo = esR.enter_context(tc.tile_pool(name="dpo", bufs=2, space="PSUM"))
            epool = esR.enter_context(tc.tile_pool(name="epool", bufs=3))

            o_ps_h = {}
            rec_h = {}

            def yblock(ch, klist, stop):
                yp = yp_ch[ch]
                for k in klist:
                    for c0, cw in ((0, 512), (512, 256)):
                        mm(
                            yp[:, c0 : c0 + cw],
                            atile[:, k, ch * 128 : (ch + 1) * 128],
                            wp_sb[:, k, c0 : c0 + cw],
                            start=(k == 0),
                            stop=(stop and k == 5),
                        )

            yp_ch = {}

            for it in range(d_heads + 2):
                # stage A (head=it-1): reciprocal of den row of o_ps;
                # broadcast the reciprocal over 64 partitions (gpsimd)
                if 0 <= it - 1 < d_heads:
                    hd = it - 1
                    o_ps = o_ps_h[hd]
                    rec = recp.tile([1, 2, 512], f32, tag="rc")
                    nc.vector.reciprocal(rec, o_ps[64:65])
                    rep = recp.tile([64, 2, 512], f32, tag="bc")
                    rec_h[hd] = rep
                    nc.gpsimd.partition_broadcast(rep, rec)

                # stage B (head=it-2): normalize straight from PSUM into atile
                if 0 <= it - 2 < d_heads:
                    hd = it - 2
                    t3 = hd // 2
                    half3 = (hd % 2) * 64
                    rep = rec_h.pop(hd)
                    o_ps = o_ps_h.pop(hd)
                    a3 = atile[half3 : half3 + 64, t3, :].rearrange(
                        "p (a b) -> p a b", a=2
                    )
                    nc.vector.tensor_mul(a3, o_ps[0:64], rep)

                # early phase E: k-blocks 0..4 only need heads 0..9, which are
                # normalized by it=11; fills the denominator-drain PE idle.
                # Only 2 chunks fit the 2 "sps" PSUM slots.
                if it == d_heads:
                    for ch in range(2):
                        yp_ch[ch] = dps.tile([128, 768], f32, tag="sps", name="yp")
                        yblock(ch, range(5), stop=False)

                # prefetch next rep's xt once the last phase-B/C consumer is done
                if it == 2 and _rep + 1 < reps:
                    nc.sync.dma_start(xt_sb[:, 0:3], xt_r[:, 0:3])
                    nc.gpsimd.dma_start(xt_sb[:, 3:6], xt_r[:, 3:6])

                # stage 0 (head=it): attention chunks
                if it < d_heads:
                    hd = it
                    par = hd % 2
                    t = hd // 2
                    o_ps = dpo.tile([65, 2, 512], f32, name="ops", tag="ops")
                    o_ps_h[hd] = o_ps

                    def o_emit(pair):
                        ch, ex = pair
                        for nt in range(2):
                            mm(
                                o_ps[:, nt],
                                vaug[:, ch, hd, :],
                                ex[:, nt * 512 : (nt + 1) * 512],
                                start=(ch == 0),
                                stop=(ch == 7),
                            )

                    # O matmuls trail the exp stream by 2 chunks so neither
                    # PE nor ACT ever waits on a fresh cross-engine handoff
                    o_q = []
                    for ch in range(8):
                        s_ps = dps.tile([128, 1024], f32, tag="sps")
                        for nt in range(2):
                            mm(
                                s_ps[:, nt * 512 : (nt + 1) * 512],
                                kasm[:, t, par, ch],
                                qaug[:, hd * 1024 + nt * 512 : hd * 1024 + (nt + 1) * 512],
                                start=True,
                                stop=True,
                            )
                        ex = expp.tile([128, 1024], bf16)
                        nc.scalar.activation(ex, s_ps, Exp)
                        o_q.append((ch, ex))
                        if len(o_q) > 2:
                            o_emit(o_q.pop(0))
                    for pair in o_q:
                        o_emit(pair)

            # ---------------- Phase E: output projection (tail) ----------------
            for ch in range(8):
                if ch < 2:
                    yblock(ch, [5], stop=True)
                else:
                    yp_ch[ch] = dps.tile([128, 768], f32, tag="sps", name="yp")
                    yblock(ch, range(6), stop=True)
                yp = yp_ch.pop(ch)
                y_sb = epool.tile([128, 768], bf16)
                nc.vector.tensor_add(y_sb, yp, bias_sb)
                nc.sync.dma_start(y[ch * 128 : (ch + 1) * 128, :], y_sb)

    nc.compile()
    return nc


def _host_prep(qkv_w, rel_pos_h, rel_pos_w, proj_w, proj_b):
    import ml_dtypes

    bf16 = ml_dtypes.bfloat16
    qkv_w = np.asarray(qkv_w, np.float32)
    scale = 1.0 / np.sqrt(HD)
    wqk = np.ascontiguousarray(qkv_w[0:1536].T)  # [768, 1536]
    wqk[:, 0:768] *= scale
    wv = np.ascontiguousarray(qkv_w[1536:2304].T).astype(bf16)  # [768, 768]
    wp = np.ascontiguousarray(np.asarray(proj_w, np.float32).T).astype(bf16)
    bias = np.ascontiguousarray(
        np.broadcast_to(np.asarray(proj_b, np.float32)[None, :], (128, 768))
    )
    k2 = np.arange(1024)
    indm = np.zeros((64, 1024), np.float32)
    indm[0:32] = (k2[None, :] // 32) == np.arange(32)[:, None]
    indm[32:64] = (k2[None, :] % 32) == np.arange(32)[:, None]
    rfh = np.ascontiguousarray(np.asarray(rel_pos_h, np.float32)[::-1].T).astype(bf16)
    rfw = np.ascontiguousarray(np.asarray(rel_pos_w, np.float32)[::-1].T).astype(bf16)
    return dict(
        wqk=wqk.astype(bf16), wv=wv, wp=wp, bias=bias, ind=indm.astype(bf16),
        rfh=rfh, rfw=rfw,
    )


def get_nc(reps=1, d_heads=12):
    key = ("nc", reps, d_heads)
    if key not in _CACHE:
        _CACHE[key] = _build_nc(reps=reps, d_heads=d_heads)
    return _CACHE[key]


def make_in_maps(x, qkv_w, rel_pos_h, rel_pos_w, proj_w, proj_b):
    import ml_dtypes

    shared = _host_prep(qkv_w, rel_pos_h, rel_pos_w, proj_w, proj_b)
    x = np.asarray(x, np.float32)
    return [
        dict(shared, xt=np.ascontiguousarray(x[b].T).astype(ml_dtypes.bfloat16))
        for b in range(x.shape[0])
    ]


def kernel(x, qkv_w, rel_pos_h, rel_pos_w, proj_w, proj_b, H=32, W=32):
    from concourse.bass_utils import run_bass_kernel_spmd

    nc = get_nc()
    in_maps = make_in_maps(x, qkv_w, rel_pos_h, rel_pos_w, proj_w, proj_b)
    res = run_bass_kernel_spmd(nc, in_maps, list(range(NCORES)))
    out = np.stack([np.asarray(res.results[b]["y"]) for b in range(NCORES)])
    return out.astype(np.float32)
